# revision 14
# baseline (speedup 1.0000x reference)
"""AFT-Full kernel for Trainium2, 8 NeuronCores.

Sharding: x [B=8, H=96, W=96, C=512] is split along H (dim 1) into 8 shards
of [8, 12, 96, 512].  Every step of the computation (q/k/v projections,
max over batch, the exp_w_bias matmuls over W, output projection) is local
to an H-slice, so there are no collectives at all.

Per-core algorithm (HL = 12 local h rows), all matmuls in bf16 with f32
PSUM accumulation:
  - x loaded pos-major ([128, 6, 512] per h; pos = b*96 + w) with an
    f32->bf16 cast during the DMA (SWDGE).
  - x transposed on the PE (bf16, via identity) into xT [c, pos] chunks.
  - qkv = x @ [wq|wk|wv]^T + bias, bias folded in as a K=1 matmul with a
    ones-row.  Output [96, 192] PSUM per (b,h).
  - exp(k), exp(-q) computed by ACT directly from PSUM (fused copy).
    sigmoid(q) = 1/(1+exp(-q)) is assembled on DVE to stay in one ACT
    table set (exp); max-over-b stabilization uses
    exp(k - mx) = exp(k) * recip(max_b exp(k)).
  - num/den = ewb^T.T @ [ekv | ek] per (b,h): [96, 128] PSUM.
  - y = num * recip(den * (1 + exp(-q))), transposed on PE to yT.
  - out = yT.T @ [out_w^T; out_b] (K=65 with a ones-row in yT), copied to
    a pos-major f32 staging tile and DMAed out per h.
"""

import sys

if "/opt/trn_rl_repo" not in sys.path:
    sys.path.insert(0, "/opt/trn_rl_repo")

import numpy as np
from contextlib import ExitStack

import concourse.bass as bass
import concourse.bacc as bacc
import concourse.tile as tile
from concourse import masks, mybir
from concourse.bass_utils import run_bass_kernel_spmd

F32 = mybir.dt.float32
BF16 = mybir.dt.bfloat16
AF = mybir.ActivationFunctionType

B = 8          # batch
S = 96         # H = W = 96
C = 512        # input channels
D = 64         # hidden
HL = 12        # h rows per core
NCORES = 8
P = 128        # partitions
T = 6          # pos-tiles of 128 per h (8*96/128)
NPOS = B * S   # 768 positions per h
BLOB_F = 1504  # packed weight blob columns

_NC_CACHE = {}


def _copy(nc, i, out, in_):
    """Alternate PSUM->SBUF copies between DVE and ACT."""
    if i % 2 == 0:
        nc.vector.tensor_copy(out, in_)
    else:
        nc.scalar.copy(out, in_)


def build_kernel():
    nc = bacc.Bacc()

    x_d = nc.declare_dram_parameter("x", [B, HL, S, C], F32, isOutput=False)
    wblob_d = nc.declare_dram_parameter("wblob", [P, BLOB_F], F32, isOutput=False)
    out_d = nc.declare_dram_parameter("out", [B, HL, S, C], F32, isOutput=True)
    xbf_d = nc.dram_tensor("xbf", [HL, B * S, C], BF16)

    with tile.TileContext(nc) as tc, ExitStack() as ctx:
        singles = ctx.enter_context(tc.tile_pool(name="singles", bufs=1))

        # ---------------- setup ----------------
        # All small weights are precomputed host-side into one [128, BLOB_F]
        # f32 blob (see _make_blob) and loaded with a single cast-DMA:
        #   cols 0:768      wqkvT   [128, 4(chunk), 192]  (q|k|v columns)
        #   cols 768:1280   owT+b   [65, 512] (row 64 = out_b)
        #   cols 1280:1376  ewbT    [96, 96]  exp(w_bias)^T
        #   cols 1376:1440  eqbB    [96, 64]  exp(-wq_b) row-replicated
        #   cols 1440:1504  vbB     [96, 64]  wv_b row-replicated
        ident = singles.tile([P, P], BF16)
        masks.make_identity(nc, ident[:])

        qkv_ps = ctx.enter_context(tc.tile_pool(name="qkv", bufs=2, space="PSUM"))
        nd_ps = ctx.enter_context(tc.tile_pool(name="nd", bufs=2, space="PSUM"))
        yt_ps = ctx.enter_context(tc.tile_pool(name="ytp", bufs=2, space="PSUM"))
        o_ps = ctx.enter_context(tc.tile_pool(name="ops", bufs=2, space="PSUM"))

        blob = singles.tile([P, BLOB_F], BF16)
        nc.gpsimd.dma_start(blob[:], wblob_d[:, :])
        wqkv = blob[:, 0:768].rearrange("p (ch x) -> p ch x", ch=4)
        ow = blob[0:D + 1, 768:1280]
        ewbT = blob[0:S, 1280:1376]
        eqbB = blob[0:S, 1376:1440]
        vbB = blob[0:S, 1440:1504]

        # ---------------- main pools ----------------
        xT_pool = ctx.enter_context(tc.tile_pool(name="xT", bufs=2))
        vsb_pool = ctx.enter_context(tc.tile_pool(name="vsb", bufs=2))
        ek_pool = ctx.enter_context(tc.tile_pool(name="ek", bufs=2))
        eq_pool = ctx.enter_context(tc.tile_pool(name="eq", bufs=2))
        small_pool = ctx.enter_context(tc.tile_pool(name="small", bufs=2))
        eks_pool = ctx.enter_context(tc.tile_pool(name="eks", bufs=2))
        teq_pool = ctx.enter_context(tc.tile_pool(name="teq", bufs=2))
        den2_pool = ctx.enter_context(tc.tile_pool(name="den2", bufs=2))
        y_pool = ctx.enter_context(tc.tile_pool(name="y", bufs=2))
        yT_pool = ctx.enter_context(tc.tile_pool(name="yT", bufs=2))
        osb_pool = ctx.enter_context(tc.tile_pool(name="osb", bufs=2))

        for h in range(HL):
            # ---- A. cast x[:, h] f32->bf16 into DRAM staging (SWDGE) ----
            nc.gpsimd.dma_start(xbf_d[h], x_d[:, h])

            # ---- B. DMA-transpose (xbar) staging -> xT [128, 4, 768] bf16 ----
            xT = xT_pool.tile([P, 4, NPOS], BF16)
            for ch in range(4):
                nc.scalar.dma_start(xT[:, ch, :],
                                    xbf_d[h][:, ch * P:(ch + 1) * P],
                                    transpose=True)

            # ---- C. qkv projection per b-pair; exps fused from PSUM ----
            ek_raw = ek_pool.tile([S, B, D], BF16)   # exp(k)
            eq = eq_pool.tile([S, B, D], BF16)       # exp(-q)
            v_sb = vsb_pool.tile([S, B, D], BF16)
            for pair in range(4):
                qp_full = qkv_ps.tile([S, 2, 4 * D], F32)
                qp = qp_full[:, :, 0:3 * D]
                for sub in range(2):
                    b = pair * 2 + sub
                    for ch in range(4):
                        nc.tensor.matmul(
                            qp[:, sub, :],
                            xT[:, ch, b * S:(b + 1) * S],
                            wqkv[:, ch, :],
                            start=(ch == 0),
                            stop=(ch == 3),
                        )
                bsl = slice(pair * 2, pair * 2 + 2)
                nc.scalar.activation(eq[:, bsl, :], qp[:, :, 0:D], AF.Exp,
                                     scale=-1.0)
                nc.scalar.activation(ek_raw[:, bsl, :], qp[:, :, D:2 * D],
                                     AF.Exp)
                nc.vector.tensor_add(v_sb[:, bsl, :], qp[:, :, 2 * D:3 * D],
                                     vbB[:, :].rearrange("p (o d) -> p o d", o=1).broadcast_to([S, 2, D]))

            # ---- D. stabilize: ek_st = ek * recip(max_b ek) ----
            mx4 = small_pool.tile([S, 4, D], BF16, tag="mx4")
            mx2 = small_pool.tile([S, 2, D], BF16, tag="mx2")
            emx = small_pool.tile([S, D], F32, tag="emx")
            remx = small_pool.tile([S, D], F32, tag="remx")
            nc.vector.tensor_max(mx4[:], ek_raw[:, 0:4, :], ek_raw[:, 4:8, :])
            nc.vector.tensor_max(mx2[:], mx4[:, 0:2, :], mx4[:, 2:4, :])
            nc.vector.tensor_max(emx[:], mx2[:, 0:1, :].rearrange("p o d -> p (o d)"),
                                 mx2[:, 1:2, :].rearrange("p o d -> p (o d)"))
            nc.vector.reciprocal_approx_fast(remx[:], emx[:])

            # eks[:, :, 0:64] = ekv = ek_st * v ; eks[:, :, 64:128] = ek_st
            eks = eks_pool.tile([S, B, 2 * D], BF16)
            nc.vector.tensor_mul(
                eks[:, :, D:2 * D], ek_raw[:],
                remx[:, :].rearrange("p (o d) -> p o d", o=1).broadcast_to([S, B, D]))
            nc.vector.tensor_mul(eks[:, :, 0:D], eks[:, :, D:2 * D], v_sb[:])

            # ---- E. num/den matmuls per b: [96(i), 128] = ewbT.T @ eks_b ----
            nd_tiles = []
            for pair in range(4):
                ndp_full = nd_ps.tile([S, 2, 4 * D], F32)
                ndp = ndp_full[:, :, 0:2 * D]
                nd_tiles.append(ndp)
                for sub in range(2):
                    b = pair * 2 + sub
                    nc.tensor.matmul(ndp[:, sub, :], ewbT[:], eks[:, b, :],
                                     start=True, stop=True)

            # ---- F. y = num * recip(den * (1 + exp(-q))) ----
            # per-pair so nd PSUM tiles release promptly (bufs=2, 4 pairs)
            teq = teq_pool.tile([S, B, D], BF16)
            nc.vector.tensor_mul(
                teq[:], eq[:],
                eqbB[:, :].rearrange("p (o d) -> p o d", o=1).broadcast_to([S, B, D]))
            nc.vector.tensor_scalar_add(teq[:], teq[:], 1.0)
            den2 = den2_pool.tile([S, B, D], F32)
            rden = den2_pool.tile([S, B, D], F32, tag="rden")
            y_sb = y_pool.tile([S, B, D], BF16)
            for pair in range(4):
                bsl = slice(pair * 2, pair * 2 + 2)
                nc.vector.tensor_mul(den2[:, bsl, :],
                                     nd_tiles[pair][:, :, D:2 * D],
                                     teq[:, bsl, :])
                nc.vector.reciprocal_approx_fast(rden[:, bsl, :], den2[:, bsl, :])
                nc.vector.tensor_mul(y_sb[:, bsl, :],
                                     nd_tiles[pair][:, :, 0:D],
                                     rden[:, bsl, :])

            # ---- G. transpose y -> yT [65, 768] bf16 (row 64 = ones) ----
            yT = yT_pool.tile([D + 1, NPOS], BF16)
            nc.vector.memset(yT[D:D + 1, :], 1.0)
            for half in range(2):
                ytp_full = yt_ps.tile([D, 1024], BF16)
                ytp = ytp_full[:, 0:384]
                for bb in range(4):
                    b = half * 4 + bb
                    nc.tensor.transpose(ytp[:, bb * S:(bb + 1) * S],
                                        y_sb[:, b, :], ident[:S, :S])
                nc.vector.tensor_copy(yT[0:D, half * 384:(half + 1) * 384], ytp[:])

            # ---- H. output projection per b + copy + DMA out ----
            o_sb = osb_pool.tile([S, B, C], F32)
            for b in range(B):
                op = o_ps.tile([S, C], F32)
                nc.tensor.matmul(op[:], yT[:, b * S:(b + 1) * S], ow[:],
                                 start=True, stop=True)
                _copy(nc, b, o_sb[:, b, :], op[:])
            dst = out_d[:, h].rearrange("b w c -> w b c")
            nc.sync.dma_start(dst, o_sb[:])

    if not nc.is_finalized():
        nc.finalize()
    return nc


def _make_blob(wq_w, wq_b, wk_w, wk_b, wv_w, wv_b, out_w, out_b, w_bias_table):
    blob = np.zeros((P, BLOB_F), dtype=np.float32)
    for j, w in enumerate([wq_w, wk_w, wv_w]):       # wqkvT [128, 4, 192]
        for ch in range(4):
            # blob[p, ch*192 + j*64 + d] = w[d, ch*128 + p]
            blob[:, ch * 192 + j * D:(ch * 192 + (j + 1) * D)] = \
                w[:, ch * P:(ch + 1) * P].T
    blob[0:D, 768:1280] = np.asarray(out_w).T        # owT
    blob[D, 768:1280] = out_b
    blob[0:S, 1280:1376] = np.exp(np.asarray(w_bias_table)).T
    # wk_b cancels exactly in exp(k - max_b k); wq_b folded via exp(-wq_b),
    # wv_b added to v after the projection.
    blob[0:S, 1376:1440] = np.exp(-np.asarray(wq_b))[None, :]
    blob[0:S, 1440:1504] = np.asarray(wv_b)[None, :]
    return blob


def kernel(x, wq_w, wq_b, wk_w, wk_b, wv_w, wv_b, out_w, out_b, w_bias_table):
    if "nc" not in _NC_CACHE:
        _NC_CACHE["nc"] = build_kernel()
    nc = _NC_CACHE["nc"]

    blob = _make_blob(wq_w, wq_b, wk_w, wk_b, wv_w, wv_b, out_w, out_b,
                      w_bias_table)
    in_maps = []
    for i in range(NCORES):
        in_maps.append({
            "wblob": blob,
            "x": np.ascontiguousarray(x[:, i * HL:(i + 1) * HL],
                                      dtype=np.float32),
        })

    res = run_bass_kernel_spmd(nc, in_maps, list(range(NCORES)))
    outs = [res.results[i]["out"] for i in range(NCORES)]
    return np.concatenate(outs, axis=1)


# revision 16
# speedup vs baseline: 2.2751x; 2.2751x over previous
"""AFT-Full kernel for Trainium2, 8 NeuronCores.

Sharding: x [B=8, H=96, W=96, C=512] is split along H (dim 1) into 8 shards
of [8, 12, 96, 512].  Every step of the computation (q/k/v projections,
max over batch, the exp_w_bias matmuls over W, output projection) is local
to an H-slice, so there are no collectives at all.

Per-core algorithm (HL = 12 local h rows), all matmuls in bf16 with f32
PSUM accumulation:
  - x loaded pos-major ([128, 6, 512] per h; pos = b*96 + w) with an
    f32->bf16 cast during the DMA (SWDGE).
  - x transposed on the PE (bf16, via identity) into xT [c, pos] chunks.
  - qkv = x @ [wq|wk|wv]^T + bias, bias folded in as a K=1 matmul with a
    ones-row.  Output [96, 192] PSUM per (b,h).
  - exp(k), exp(-q) computed by ACT directly from PSUM (fused copy).
    sigmoid(q) = 1/(1+exp(-q)) is assembled on DVE to stay in one ACT
    table set (exp); max-over-b stabilization uses
    exp(k - mx) = exp(k) * recip(max_b exp(k)).
  - num/den = ewb^T.T @ [ekv | ek] per (b,h): [96, 128] PSUM.
  - y = num * recip(den * (1 + exp(-q))), transposed on PE to yT.
  - out = yT.T @ [out_w^T; out_b] (K=65 with a ones-row in yT), copied to
    a pos-major f32 staging tile and DMAed out per h.
"""

import sys

if "/opt/trn_rl_repo" not in sys.path:
    sys.path.insert(0, "/opt/trn_rl_repo")

import numpy as np
from contextlib import ExitStack

import concourse.bass as bass
import concourse.bacc as bacc
import concourse.tile as tile
from concourse import masks, mybir
from concourse.bass_utils import run_bass_kernel_spmd

F32 = mybir.dt.float32
BF16 = mybir.dt.bfloat16
AF = mybir.ActivationFunctionType

B = 8          # batch
S = 96         # H = W = 96
C = 512        # input channels
D = 64         # hidden
HL = 12        # h rows per core
NCORES = 8
P = 128        # partitions
T = 6          # pos-tiles of 128 per h (8*96/128)
NPOS = B * S   # 768 positions per h
BLOB_F = 1504  # packed weight blob columns

_NC_CACHE = {}


def _copy(nc, i, out, in_):
    """Alternate PSUM->SBUF copies between DVE and ACT."""
    if i % 2 == 0:
        nc.vector.tensor_copy(out, in_)
    else:
        nc.scalar.copy(out, in_)


def build_kernel():
    nc = bacc.Bacc()

    x_d = nc.declare_dram_parameter("x", [B, HL, S, C], F32, isOutput=False)
    wblob_d = nc.declare_dram_parameter("wblob", [P, BLOB_F], F32, isOutput=False)
    out_d = nc.declare_dram_parameter("out", [B, HL, S, C], F32, isOutput=True)

    with tile.TileContext(nc) as tc, ExitStack() as ctx:
        singles = ctx.enter_context(tc.tile_pool(name="singles", bufs=1))

        # ---------------- setup ----------------
        # All small weights are precomputed host-side into one [128, BLOB_F]
        # f32 blob (see _make_blob) and loaded with a single cast-DMA:
        #   cols 0:768      wqkvT   [128, 4(chunk), 192]  (q|k|v columns)
        #   cols 768:1280   owT+b   [65, 512] (row 64 = out_b)
        #   cols 1280:1376  ewbT    [96, 96]  exp(w_bias)^T
        #   cols 1376:1440  eqbB    [96, 64]  exp(-wq_b) row-replicated
        #   cols 1440:1504  vbB     [96, 64]  wv_b row-replicated
        ident = singles.tile([P, P], BF16)
        masks.make_identity(nc, ident[:])

        xtp_ps = ctx.enter_context(tc.tile_pool(name="xtp", bufs=2, space="PSUM"))
        qkv_ps = ctx.enter_context(tc.tile_pool(name="qkv", bufs=2, space="PSUM"))
        nd_ps = ctx.enter_context(tc.tile_pool(name="nd", bufs=2, space="PSUM"))
        yt_ps = ctx.enter_context(tc.tile_pool(name="ytp", bufs=1, space="PSUM"))
        o_ps = ctx.enter_context(tc.tile_pool(name="ops", bufs=1, space="PSUM"))

        blob = singles.tile([P, BLOB_F], BF16)
        nc.gpsimd.dma_start(blob[:], wblob_d[:, :])
        wqkv = blob[:, 0:768].rearrange("p (ch x) -> p ch x", ch=4)
        ow = blob[0:D + 1, 768:1280]
        ewbT = blob[0:S, 1280:1376]
        eqbB = blob[0:S, 1376:1440]
        vbB = blob[0:S, 1440:1504]

        # ---------------- main pools ----------------
        xnat_pool = ctx.enter_context(tc.tile_pool(name="xnat", bufs=2))
        xT_pool = ctx.enter_context(tc.tile_pool(name="xT", bufs=2))
        vsb_pool = ctx.enter_context(tc.tile_pool(name="vsb", bufs=2))
        ek_pool = ctx.enter_context(tc.tile_pool(name="ek", bufs=2))
        eq_pool = ctx.enter_context(tc.tile_pool(name="eq", bufs=2))
        small_pool = ctx.enter_context(tc.tile_pool(name="small", bufs=2))
        eks_pool = ctx.enter_context(tc.tile_pool(name="eks", bufs=2))
        teq_pool = ctx.enter_context(tc.tile_pool(name="teq", bufs=2))
        den2_pool = ctx.enter_context(tc.tile_pool(name="den2", bufs=2))
        y_pool = ctx.enter_context(tc.tile_pool(name="y", bufs=2))
        yT_pool = ctx.enter_context(tc.tile_pool(name="yT", bufs=2))
        osb_pool = ctx.enter_context(tc.tile_pool(name="osb", bufs=2))

        yT_tiles = [yT_pool.tile([D + 1, NPOS], BF16, tag=f"yt{i}",
                                 name=f"yt{i}") for i in range(2)]
        for t in yT_tiles:
            nc.vector.memset(t[D:D + 1, :], 1.0)

        for h in range(HL):
            # ---- A. load x[:, h] w-major with f32->bf16 cast ----
            x_nat = xnat_pool.tile([S, B, C], BF16)
            src = x_d[:, h].rearrange("b w c -> w b c")
            nc.gpsimd.dma_start(x_nat[:], src)

            # ---- B. transpose x -> xT [128(c), 4(chunk), 768(pos)] bf16 ----
            # pos = b*96 + w is linear in the free dim of xT.
            xT = xT_pool.tile([P, 4, NPOS], BF16)
            ci = 0
            for ch in range(4):
                for half in range(2):
                    tp_full = xtp_ps.tile([P, 1024], BF16, tag="tp")
                    tp = tp_full[:, 0:384]
                    for bb in range(4):
                        b = half * 4 + bb
                        nc.tensor.transpose(
                            tp[:, bb * S:(bb + 1) * S],
                            x_nat[:, b, ch * P:(ch + 1) * P],
                            ident[:S, :S],
                        )
                    _copy(nc, ci, xT[:, ch, half * 384:(half + 1) * 384], tp[:])
                    ci += 1

            # ---- C. qkv projection per b-pair; exps fused from PSUM ----
            ek_raw = ek_pool.tile([S, B, D], BF16)   # exp(k)
            eq = eq_pool.tile([S, B, D], BF16)       # exp(-q)
            v_sb = vsb_pool.tile([S, B, D], BF16)
            for pair in range(4):
                qp_full = qkv_ps.tile([S, 2, 4 * D], F32)
                qp = qp_full[:, :, 0:3 * D]
                for sub in range(2):
                    b = pair * 2 + sub
                    for ch in range(4):
                        nc.tensor.matmul(
                            qp[:, sub, :],
                            xT[:, ch, b * S:(b + 1) * S],
                            wqkv[:, ch, :],
                            start=(ch == 0),
                            stop=(ch == 3),
                        )
                bsl = slice(pair * 2, pair * 2 + 2)
                nc.scalar.activation(eq[:, bsl, :], qp[:, :, 0:D], AF.Exp,
                                     scale=-1.0)
                nc.scalar.activation(ek_raw[:, bsl, :], qp[:, :, D:2 * D],
                                     AF.Exp)
                nc.vector.tensor_add(v_sb[:, bsl, :], qp[:, :, 2 * D:3 * D],
                                     vbB[:, :].rearrange("p (o d) -> p o d", o=1).broadcast_to([S, 2, D]))

            # ---- D. stabilize: ek_st = ek * recip(max_b ek) ----
            mx4 = small_pool.tile([S, 4, D], BF16, tag="mx4")
            mx2 = small_pool.tile([S, 2, D], BF16, tag="mx2")
            emx = small_pool.tile([S, D], F32, tag="emx")
            remx = small_pool.tile([S, D], F32, tag="remx")
            nc.vector.tensor_max(mx4[:], ek_raw[:, 0:4, :], ek_raw[:, 4:8, :])
            nc.vector.tensor_max(mx2[:], mx4[:, 0:2, :], mx4[:, 2:4, :])
            nc.vector.tensor_max(emx[:], mx2[:, 0:1, :].rearrange("p o d -> p (o d)"),
                                 mx2[:, 1:2, :].rearrange("p o d -> p (o d)"))
            nc.vector.reciprocal_approx_fast(remx[:], emx[:])

            # eks[:, :, 0:64] = ekv = ek_st * v ; eks[:, :, 64:128] = ek_st
            eks = eks_pool.tile([S, B, 2 * D], BF16)
            nc.vector.tensor_mul(
                eks[:, :, D:2 * D], ek_raw[:],
                remx[:, :].rearrange("p (o d) -> p o d", o=1).broadcast_to([S, B, D]))
            nc.vector.tensor_mul(eks[:, :, 0:D], eks[:, :, D:2 * D], v_sb[:])

            # ---- E. num/den matmuls per b: [96(i), 128] = ewbT.T @ eks_b ----
            nd_tiles = []
            for pair in range(4):
                ndp_full = nd_ps.tile([S, 2, 4 * D], F32)
                ndp = ndp_full[:, :, 0:2 * D]
                nd_tiles.append(ndp)
                for sub in range(2):
                    b = pair * 2 + sub
                    nc.tensor.matmul(ndp[:, sub, :], ewbT[:], eks[:, b, :],
                                     start=True, stop=True)

            # ---- F. y = num * recip(den * (1 + exp(-q))) ----
            # per-pair so nd PSUM tiles release promptly (bufs=2, 4 pairs)
            teq = teq_pool.tile([S, B, D], BF16)
            nc.vector.tensor_mul(
                teq[:], eq[:],
                eqbB[:, :].rearrange("p (o d) -> p o d", o=1).broadcast_to([S, B, D]))
            nc.vector.tensor_scalar_add(teq[:], teq[:], 1.0)
            den2 = den2_pool.tile([S, B, D], F32)
            rden = den2_pool.tile([S, B, D], F32, tag="rden")
            y_sb = y_pool.tile([S, B, D], BF16)
            for pair in range(4):
                bsl = slice(pair * 2, pair * 2 + 2)
                nc.vector.tensor_mul(den2[:, bsl, :],
                                     nd_tiles[pair][:, :, D:2 * D],
                                     teq[:, bsl, :])
                nc.vector.reciprocal_approx_fast(rden[:, bsl, :], den2[:, bsl, :])
                nc.vector.tensor_mul(y_sb[:, bsl, :],
                                     nd_tiles[pair][:, :, 0:D],
                                     rden[:, bsl, :])

            # ---- G. transpose y -> yT [65, 768] bf16 (row 64 = ones) ----
            yT = yT_tiles[h % 2]
            for half in range(2):
                ytp_full = yt_ps.tile([D, 1024], BF16)
                ytp = ytp_full[:, 0:384]
                for bb in range(4):
                    b = half * 4 + bb
                    nc.tensor.transpose(ytp[:, bb * S:(bb + 1) * S],
                                        y_sb[:, b, :], ident[:S, :S])
                nc.vector.tensor_copy(yT[0:D, half * 384:(half + 1) * 384], ytp[:])

            # ---- H. output projection per b + copy + DMA out ----
            o_sb = osb_pool.tile([S, B, C], F32)
            for b in range(B):
                op = o_ps.tile([S, C], F32)
                nc.tensor.matmul(op[:], yT[:, b * S:(b + 1) * S], ow[:],
                                 start=True, stop=True)
                _copy(nc, b, o_sb[:, b, :], op[:])
            dst = out_d[:, h].rearrange("b w c -> w b c")
            nc.sync.dma_start(dst, o_sb[:])

    if not nc.is_finalized():
        nc.finalize()
    return nc


def _make_blob(wq_w, wq_b, wk_w, wk_b, wv_w, wv_b, out_w, out_b, w_bias_table):
    blob = np.zeros((P, BLOB_F), dtype=np.float32)
    for j, w in enumerate([wq_w, wk_w, wv_w]):       # wqkvT [128, 4, 192]
        for ch in range(4):
            # blob[p, ch*192 + j*64 + d] = w[d, ch*128 + p]
            blob[:, ch * 192 + j * D:(ch * 192 + (j + 1) * D)] = \
                w[:, ch * P:(ch + 1) * P].T
    blob[0:D, 768:1280] = np.asarray(out_w).T        # owT
    blob[D, 768:1280] = out_b
    blob[0:S, 1280:1376] = np.exp(np.asarray(w_bias_table)).T
    # wk_b cancels exactly in exp(k - max_b k); wq_b folded via exp(-wq_b),
    # wv_b added to v after the projection.
    blob[0:S, 1376:1440] = np.exp(-np.asarray(wq_b))[None, :]
    blob[0:S, 1440:1504] = np.asarray(wv_b)[None, :]
    return blob


def kernel(x, wq_w, wq_b, wk_w, wk_b, wv_w, wv_b, out_w, out_b, w_bias_table):
    if "nc" not in _NC_CACHE:
        _NC_CACHE["nc"] = build_kernel()
    nc = _NC_CACHE["nc"]

    blob = _make_blob(wq_w, wq_b, wk_w, wk_b, wv_w, wv_b, out_w, out_b,
                      w_bias_table)
    in_maps = []
    for i in range(NCORES):
        in_maps.append({
            "wblob": blob,
            "x": np.ascontiguousarray(x[:, i * HL:(i + 1) * HL],
                                      dtype=np.float32),
        })

    res = run_bass_kernel_spmd(nc, in_maps, list(range(NCORES)))
    outs = [res.results[i]["out"] for i in range(NCORES)]
    return np.concatenate(outs, axis=1)
